# revision 3
# baseline (speedup 1.0000x reference)
"""nn_NSLayer kernel for 8 Trainium2 NeuronCores.

Computation (per 16x16 matrix X, batched over [64, 512]):
    A = I - X @ X.T
    mats = [A, A^2, A^4, ..., A^(2^13)]      (repeated squaring)
    Mat = sum_l |w_l| * mats[l]
    out = X + Mat @ X

Sharding: pure data parallel over the batch dim B (64 -> 8 per core);
weights replicated; no cross-device communication. Each core runs the
faithful fp32 chain on its [8, 512, 16, 16] shard via XLA-on-neuron.

Numerical note: with X ~ N(0,1) the squaring chain overflows fp32 at
A^32 for every matrix, so the reference output is entirely Inf/NaN and
its class pattern is governed by exact sign propagation, which this
faithful same-order fp32 implementation reproduces.
"""

import numpy as np

B, C, K = 64, 512, 16
N_CORES = 8
N_TERMS = 14

_cache = {}


def _get_fn():
    if "fn" in _cache:
        return _cache["fn"]
    import jax
    import jax.numpy as jnp

    def shard_fn(x, w):
        # x: [B/8, C, K, K] on one core
        eye = jnp.eye(K, dtype=x.dtype)
        a = eye - jnp.einsum('bcij,bckj->bcik', x, x)
        mats = [a]
        cur = a
        for _ in range(N_TERMS - 1):
            cur = jnp.matmul(cur, cur)
            mats.append(cur)
        wabs = jnp.abs(w)
        stacked = jnp.stack(mats, axis=0)
        mat = jnp.einsum('n,nbcik->bcik', wabs, stacked)
        return x + jnp.matmul(mat, x)

    fn = jax.pmap(shard_fn, in_axes=(0, None), devices=jax.devices()[:N_CORES])
    _cache["fn"] = fn
    return fn


def kernel(input, weight):
    import jax.numpy as jnp

    X = np.asarray(input, dtype=np.float32)
    assert X.shape == (B, C, K, K)
    Xs = X.reshape(N_CORES, B // N_CORES, C, K, K)
    w = jnp.asarray(np.asarray(weight, dtype=np.float32))
    fn = _get_fn()
    out = fn(Xs, w)
    return np.asarray(out).reshape(B, C, K, K).astype(np.float32)
